# revision 1
# baseline (speedup 1.0000x reference)
"""LRN (Local Response Normalization, TF-style cross-W+C window) Trainium2 kernel.

Reference computation (on [B,H,W,C] = [32,224,224,64] f32):
    s[b,h,w]   = sum_c x[b,h,w,c]^2
    win[b,h,w] = sum_{d=-5..5} s[b,h,w+d]        (zero-padded SAME over W)
    out        = x / sqrt(1 + 1.0*win)           (bias=1, alpha=1, beta=0.5)

Sharding: pure data-parallel over batch. 8 cores x 4 batches each.
Per-core layout: rows = (b,h) pairs -> 896 rows = 7 tiles of 128 partitions,
free axis = (w, c) = 224*64 = 14336 f32 (57 KiB/partition, contiguous in HBM).

Engine balance per tile (cost-model estimates, bf16 output):
  DMA  in 4x5.1us (f32 w-chunks, SP queue) + out 4x2.5us (bf16 w-chunks,
       ACT queue)                             -> the bottleneck (30.6us)
  ACT  Square (4 chunks of 56 w)    ~12.8us
  DVE  reduce_sum axis=X (4 chunks) ~15.2us   (grouped C-sum), recip 0.3us,
       2 final-mul chunks 2x3.8us
  Pool window log-shift adds ~2.8us, 2 final-mul chunks 2x7.2us
  ACT  Sqrt(alpha*win + bias)       ~0.4us

The output is written as bfloat16 (max ~4e-3 relative rounding error vs the
2e-2 gate) to cut output DMA traffic in half; all arithmetic is f32.
Input and output DMAs use different HWDGE queues (SP vs ACT) so an output
chunk waiting on its multiply cannot block the next tile's input DMAs.
"""

import json
import re

import numpy as np

import concourse.bass as bass
import concourse.tile as tile
from concourse import mybir
from concourse.bass_utils import run_bass_kernel_spmd

# Problem constants (hardcoded per harness contract).
B, H, W, C = 32, 224, 224, 64
N_CORES = 8
RADIUS = 5
KWIN = 2 * RADIUS + 1  # 11
BIAS = 1.0
ALPHA = 1.0

P = 128
B_PER_CORE = B // N_CORES          # 4
ROWS = B_PER_CORE * H              # 896
NTILES = ROWS // P                 # 7
N_WCHUNK = 4
WCH = W // N_WCHUNK                # 56
WPAD = W + KWIN - 1                # 234

_F32 = mybir.dt.float32

# Output dtype for the DRAM result. float32 matches reference bit-for-bit
# precision expectations; bfloat16 halves the output DMA traffic at ~4e-3
# max relative error (well inside the 2e-2 gate).
_BF16 = mybir.dt.bfloat16

OUT_DTYPE = _BF16

# Squared-values buffer dtype. bfloat16 halves the reduce input bytes and
# can enable the DVE 2x perf mode on the C-reduction; costs ~4e-3 relative
# error on the window sum (gate is 2e-2).
X2_DTYPE = _F32

# How many of the 4 final-multiply w-chunks run on the Pool engine instead
# of DVE (load balance: DVE also owns the C-reduction).
MUL_POOL_CHUNKS = 2

# Run the 5 sliding-window adds on Pool instead of DVE.
WINDOW_ON_POOL = True

# Issue output DMAs from the Activation engine's HWDGE queue instead of
# SP. A pending output DMA (whose mul has not finished) then cannot block
# the next tile's input DMAs behind it in the SP queue.
OUT_DMA_ON_ACT = True

# Tile-pool depths (chunks): xpool holds input w-chunks (4 per tile),
# opool holds output w-chunks.
XPOOL_BUFS = 8
OPOOL_BUFS = 4

# Split the C-reduction: Pool adds the two C-halves (x2[..,0:32]+x2[..,32:64])
# and DVE reduces the half-width result. Halves the serialized DVE reduce
# chain that otherwise gates the rstd latency.
HALVE_ADD = False

# The walrus build in this container accepts only ONE sync-wait slot per TPB
# instruction ("Too many sync wait commands" in setupSyncWait otherwise),
# while Tile's scheduler freely attaches 2-3 waits per instruction. Legalize
# the BIR before compilation: drop same-engine program-order self-waits
# (trivially satisfied on an in-order sequencer) and hoist any remaining
# excess waits onto standalone EventSemaphore instructions just before the
# owning instruction on the same engine.
_ENGINE_SEM = re.compile(r"^(Pool|Activation|PE|DVE|SP)_\d+$")


def _legalize_bir_waits(bir: bytes, max_waits: int = 1) -> bytes:
    d = json.loads(bir)
    incers: dict = {}
    for fn in d["functions"]:
        for bb in fn.get("blocks") or []:
            for ins in bb["instructions"]:
                for u in (ins.get("sync_info") or {}).get("on_update") or []:
                    incers.setdefault(u["id"], set()).add(
                        (ins.get("engine"), ins.get("opcode"))
                    )
    n_ev = 0
    for fn in d["functions"]:
        for bb in fn.get("blocks") or []:
            out = []
            for ins in bb["instructions"]:
                si = ins.get("sync_info")
                waits = (si or {}).get("on_wait") or []
                opcode = ins.get("opcode")
                if (
                    si
                    and len(waits) > max_waits
                    and opcode != "EventSemaphore"
                ):
                    eng = ins.get("engine")
                    kept = []
                    for w in waits:
                        nm = w.get("ant_name", "")
                        srcs = incers.get(w.get("id"), set())
                        if (
                            _ENGINE_SEM.match(nm)
                            and nm.startswith(str(eng) + "_")
                            and srcs
                            and all(
                                e == eng and op != "DMACopy" for e, op in srcs
                            )
                        ):
                            # Same-engine program-order wait: every inc comes
                            # from an earlier instruction on this in-order
                            # engine, so it holds by the time this issues.
                            continue
                        kept.append(w)
                    for w in kept[max_waits:]:
                        n_ev += 1
                        out.append(
                            {
                                "debug": ins.get("debug", 0),
                                "engine": eng,
                                "ins": [],
                                "outs": [],
                                "name": f"evw-{n_ev}",
                                "opcode": "EventSemaphore",
                                "sync_info": {"on_update": [], "on_wait": [w]},
                            }
                        )
                    si["on_wait"] = kept[:max_waits]
                out.append(ins)
            bb["instructions"] = out
    return json.dumps(d).encode()


class _WaitLegalBass(bass.Bass):
    def to_json_bytes(self) -> bytes:
        return _legalize_bir_waits(super().to_json_bytes())


def _bcast_c(ap: bass.AP) -> bass.AP:
    """Broadcast a [P, n] AP over a trailing C axis via stride 0."""
    return bass.AP(
        tensor=ap.tensor,
        offset=ap.offset,
        ap=[ap.ap[0], ap.ap[1], [0, C]],
    )


def build_nc(
    out_dtype=OUT_DTYPE, chain: int = 1, loop_iters: int = 0
) -> bass.Bass:
    """Build the LRN kernel.

    chain > 1 repeats the identical full pass (same x -> same y) inline.
    loop_iters > 0 additionally wraps the `chain` inline passes in a
    tc.For_i hardware loop executed loop_iters times, so one NEFF runs
    chain*loop_iters passes with a small instruction stream. Used by bench()
    to make on-device time dominate the noisy per-call dispatch overhead.
    """
    import contextlib

    nc = _WaitLegalBass(trn_type="TRN2")
    x = nc.dram_tensor("x", [ROWS, W, C], _F32, kind="ExternalInput")
    y = nc.dram_tensor("y", [ROWS, W, C], out_dtype, kind="ExternalOutput")

    with tile.TileContext(nc) as tc:
        with (
            tc.tile_pool(name="xpool", bufs=XPOOL_BUFS) as xpool,
            tc.tile_pool(name="x2pool", bufs=2) as x2pool,
            tc.tile_pool(name="xhpool", bufs=2) as xhpool,
            tc.tile_pool(name="opool", bufs=OPOOL_BUFS) as opool,
            tc.tile_pool(name="spool", bufs=2) as spool,
            tc.tile_pool(name="wpool", bufs=2) as wpool,
        ):

            def emit_pass():
                for it in range(NTILES):
                    r0 = it * P
                    # Input arrives as 4 w-chunks so squares/reduces start
                    # as soon as each chunk lands (finer DMA interleave also
                    # avoids head-of-line blocking of the next tile's input
                    # behind this tile's output chunks).
                    x_chunks = []
                    for jc in range(N_WCHUNK):
                        w0 = jc * WCH
                        xc = xpool.tile([P, WCH, C], _F32)
                        nc.sync.dma_start(
                            out=xc, in_=x[r0 : r0 + P, w0 : w0 + WCH, :]
                        )
                        x_chunks.append(xc)

                    # s_pad holds the C-sums, 5-wide zero border each side.
                    s_pad = spool.tile([P, WPAD], _F32)
                    nc.gpsimd.memset(s_pad[:, 0:RADIUS], 0.0)
                    nc.gpsimd.memset(s_pad[:, W + RADIUS : WPAD], 0.0)

                    for jc in range(N_WCHUNK):
                        w0 = jc * WCH
                        x2 = x2pool.tile([P, WCH, C], X2_DTYPE)
                        # Square on ACT; grouped C-sum on DVE (axis=X
                        # reduction is DVE-only).
                        nc.scalar.square(x2, x_chunks[jc])
                        if HALVE_ADD:
                            xh = xhpool.tile([P, WCH, C // 2], _F32)
                            nc.gpsimd.scalar_tensor_tensor(
                                out=xh,
                                in0=x2[:, :, 0 : C // 2],
                                scalar=1.0,
                                in1=x2[:, :, C // 2 : C],
                                op0=mybir.AluOpType.mult,
                                op1=mybir.AluOpType.add,
                            )
                            nc.vector.reduce_sum(
                                out=s_pad[:, RADIUS + w0 : RADIUS + w0 + WCH],
                                in_=xh,
                                axis=mybir.AxisListType.X,
                            )
                        else:
                            nc.vector.reduce_sum(
                                out=s_pad[:, RADIUS + w0 : RADIUS + w0 + WCH],
                                in_=x2,
                                axis=mybir.AxisListType.X,
                            )

                    # Sliding-window sum of width 11 via log-shift adds.
                    # win[w] = sum_{d=0..10} s_pad[w+d],  w in [0, 224).
                    weng = nc.gpsimd if WINDOW_ON_POOL else nc.vector
                    w2 = wpool.tile([P, WPAD - 1], _F32)
                    weng.tensor_add(
                        w2, s_pad[:, 0 : WPAD - 1], s_pad[:, 1:WPAD]
                    )
                    w4 = wpool.tile([P, WPAD - 3], _F32)
                    weng.tensor_add(
                        w4, w2[:, 0 : WPAD - 3], w2[:, 2 : WPAD - 1]
                    )
                    w8 = wpool.tile([P, WPAD - 7], _F32)
                    weng.tensor_add(
                        w8, w4[:, 0 : WPAD - 7], w4[:, 4 : WPAD - 3]
                    )
                    t10 = wpool.tile([P, W], _F32)
                    weng.tensor_add(t10, w8[:, 0:W], w2[:, 8 : 8 + W])
                    win = wpool.tile([P, W], _F32)
                    weng.tensor_add(win, t10, s_pad[:, 10 : 10 + W])

                    # denom = sqrt(alpha*win + bias); rstd = 1/denom.
                    denom = wpool.tile([P, W], _F32)
                    nc.scalar.activation(
                        out=denom,
                        in_=win,
                        func=mybir.ActivationFunctionType.Sqrt,
                        bias=BIAS,
                        scale=ALPHA,
                    )
                    rstd = wpool.tile([P, W], _F32)
                    nc.vector.reciprocal(out=rstd, in_=denom)

                    # out = x * rstd broadcast over C, chunked so output
                    # DMAs start early and overlap the next tile's input
                    # DMA. A couple of chunks go to the Pool engine to keep
                    # DVE under the DMA roofline.
                    for jc in range(N_WCHUNK):
                        w0 = jc * WCH
                        out_c = opool.tile([P, WCH, C], out_dtype)
                        eng = (
                            nc.gpsimd if jc < MUL_POOL_CHUNKS else nc.vector
                        )
                        eng.tensor_mul(
                            out_c,
                            x_chunks[jc],
                            _bcast_c(rstd[:, w0 : w0 + WCH]),
                        )
                        dma_eng = nc.scalar if OUT_DMA_ON_ACT else nc.sync
                        dma_eng.dma_start(
                            out=y[r0 : r0 + P, w0 : w0 + WCH, :], in_=out_c
                        )

            loop_cm = (
                tc.For_i(0, loop_iters)
                if loop_iters > 0
                else contextlib.nullcontext()
            )
            with loop_cm:
                for _rep in range(chain):
                    emit_pass()

    return nc


_NC_CACHE: dict = {}


def _get_nc(chain: int = 1, loop_iters: int = 0) -> bass.Bass:
    key = (OUT_DTYPE, chain, loop_iters)
    if key not in _NC_CACHE:
        _NC_CACHE[key] = build_nc(OUT_DTYPE, chain, loop_iters)
    return _NC_CACHE[key]


def run(x: np.ndarray, **kwargs):
    """Run the SPMD kernel on 8 cores. Returns (out, BassKernelResults)."""
    x = np.ascontiguousarray(x, dtype=np.float32)
    assert x.shape == (B, H, W, C)
    nc = _get_nc()
    in_maps = [
        {"x": x[i * B_PER_CORE : (i + 1) * B_PER_CORE].reshape(ROWS, W, C)}
        for i in range(N_CORES)
    ]
    res = run_bass_kernel_spmd(nc, in_maps, core_ids=list(range(N_CORES)), **kwargs)
    outs = [
        r["y"].astype(np.float32).reshape(B_PER_CORE, H, W, C)
        for r in res.results
    ]
    out = np.concatenate(outs, axis=0)
    return out, res


def kernel(x: np.ndarray) -> np.ndarray:
    out, _ = run(x)
    return out


def _make_fn(nc):
    """jax.jit-wrapped single bass_exec call over an 8-core mesh."""
    import jax
    from jax.sharding import Mesh, PartitionSpec
    from jax.experimental.shard_map import shard_map

    from concourse import bass2jax
    from concourse import mybir as _mybir

    bass2jax.install_neuronx_cc_hook()

    partition_name = (
        nc.partition_id_tensor.name if nc.partition_id_tensor is not None else None
    )
    in_names, out_names, out_avals = [], [], []
    for alloc in nc.m.functions[0].allocations:
        if not isinstance(alloc, _mybir.MemoryLocationSet):
            continue
        name = alloc.memorylocations[0].name
        if alloc.kind == "ExternalInput":
            if name != partition_name:
                in_names.append(name)
        elif alloc.kind == "ExternalOutput":
            out_names.append(name)
            out_avals.append(
                jax.core.ShapedArray(
                    tuple(alloc.tensor_shape), _mybir.dt.np(alloc.dtype)
                )
            )
    n_params = len(in_names)
    all_names = in_names + out_names
    if partition_name is not None:
        all_names = all_names + [partition_name]

    def _body(*args):
        operands = list(args)
        if partition_name is not None:
            operands.append(bass2jax.partition_id_tensor())
        outs = bass2jax._bass_exec_p.bind(
            *operands,
            out_avals=tuple(out_avals),
            in_names=tuple(all_names),
            out_names=tuple(out_names),
            lowering_input_output_aliases=(),
            sim_require_finite=True,
            sim_require_nnan=True,
            nc=nc,
        )
        return tuple(outs)

    devices = jax.devices()[:N_CORES]
    mesh = Mesh(np.asarray(devices), ("core",))
    nspec = n_params + len(out_names)
    fn = jax.jit(
        shard_map(
            _body,
            mesh=mesh,
            in_specs=(PartitionSpec("core"),) * nspec,
            out_specs=(PartitionSpec("core"),) * len(out_names),
            check_rep=False,
        ),
        keep_unused=True,
    )
    return fn, out_avals, mesh


def bench(
    x: np.ndarray,
    reps: int = 13,
    warmup: int = 2,
) -> dict:
    """Measure steady-state device time per LRN pass.

    Three NEFFs, each a tc.For_i hardware loop over a body of identical
    full passes (same x -> same y):

      S: body=2 passes, 4 iters   (8 passes,   4 loop barriers)
      A: body=2 passes, 64 iters  (128 passes, 64 loop barriers)
      B: body=4 passes, 32 iters  (128 passes, 32 loop barriers)

    Per-call wall time = dispatch + passes*steady + barriers*X. Solving the
    pairwise differences cancels the noisy axon dispatch overhead and the
    per-iteration barrier cost X (a benchmark-harness artifact: the For_i
    reset drains the tile pipeline, which back-to-back kernel invocations
    do not do):

      X      = (T_A - T_B) / 32
      steady = ((T_A - T_S) - 60*X) / 120

    `steady` is reported as the per-pass device time.
    """
    import time

    import jax
    from jax.sharding import Mesh, PartitionSpec

    x = np.ascontiguousarray(x, dtype=np.float32)
    cfgs = {"S": (2, 4), "A": (2, 64), "B": (4, 32)}
    fns = {}
    out_avals = mesh = None
    for k, (body, iters) in cfgs.items():
        nck = _get_nc(chain=body, loop_iters=iters)
        fn, avals, m = _make_fn(nck)
        fns[k] = fn
        if out_avals is None:
            out_avals, mesh = avals, m

    xg = x.reshape(N_CORES * ROWS, W, C)
    zeros = [
        np.zeros((N_CORES * a.shape[0],) + tuple(a.shape[1:]), a.dtype)
        for a in out_avals
    ]
    sharding = jax.sharding.NamedSharding(mesh, PartitionSpec("core"))
    dev_args = [jax.device_put(a, sharding) for a in [xg] + zeros]

    outS = None
    for _ in range(warmup):
        for k, f in fns.items():
            o = f(*dev_args)
            jax.block_until_ready(o)
            if k == "S":
                outS = o

    # Round-robin the three NEFFs so terminal latency drift (the dominant
    # noise, ~10ms quantized) hits all of them equally.
    samples = {k: [] for k in fns}
    for _ in range(reps):
        for k in ("S", "A", "B"):
            t0 = time.perf_counter()
            jax.block_until_ready(fns[k](*dev_args))
            samples[k].append(time.perf_counter() - t0)
    T = {k: min(v) for k, v in samples.items()}

    # Barrier cost per For_i iteration. The A-B contrast is a weak signal
    # (~1ms) vs the ~10ms-quantized RPC noise, so clamp to [0, plausible]
    # rather than letting a bad draw corrupt the steady estimate.
    X = min(max((T["A"] - T["B"]) / 32.0, 0.0), 100e-6)
    # Steady per-pass time from both 120-pass contrasts (A carries 60X of
    # barrier overhead, B carries 28X); take the smaller, which is still
    # conservative when X is underestimated (overhead stays included).
    steady = min(
        ((T["A"] - T["S"]) - 60.0 * X) / 120.0,
        ((T["B"] - T["S"]) - 28.0 * X) / 120.0,
    )
    device_ns = steady * 1e9

    result = (
        np.asarray(outS[0]).astype(np.float32).reshape(B, H, W, C)
    )
    return {
        "device_ns": device_ns,
        "t1_ns": T["S"] * 1e9,
        "tN_ns": T["A"] * 1e9,
        "n_chain": 120,
        "barrier_ns": X * 1e9,
        "samples_ms": {
            k: [round(t * 1e3, 1) for t in v] for k, v in samples.items()
        },
        "out": result,
    }

